# revision 11
# baseline (speedup 1.0000x reference)
"""Trainium2 Bass kernel for nn_EncodingModule2d (vq_codebook).

Pipeline per batch item (pure data parallel, 1 item per NeuronCore, 8 cores):
  stem:   s = conv_w @ x  (1x1 conv as 256x256 matmul over 4096 positions)
          y = relu(BN2(s))                          -- BN folded into weights on host
  vq:     dist2[n,k] = |y_n|^2 - 2<y_n, c_k> + |c_k|^2
          a = softmax_k(scales_k * dist2)
          agg[k,:] = sum_n a[n,k] (y_n - c_k)
  post:   z = mean_k relu(BN1(agg))                 -- BN folded on host
          g = sigmoid(head_w @ z + head_b)
  out:    relu(x + x * g) = relu(x * (1 + g))

The kernel computes the stem in BOTH (d,n) and (n,d) layouts directly from x
(two matmul orientations) because the distance matmul contracts over d while
the aggregation matmul contracts over n; this costs the same PE time as one
stem plus PE transposes but avoids ~20us of PSUM->SBUF copy traffic.

dtype strategy: float32r (1 cyc/row on the PE when N>=256, vs 4 for float32)
for the stem and aggregation matmuls; plain float32 for the N=32 distance
matmuls where fp32r has no speed advantage anyway (and would force rounding
of y). fp32r matmuls require even N, hence the 258-wide aggregation rhs
(256 y columns + ones column + dummy pad column).
"""

import os
import sys

for _p in ("/opt/trn_rl_repo",):
    if _p not in sys.path and os.path.isdir(_p):
        sys.path.insert(0, _p)

from contextlib import ExitStack

import numpy as np

import concourse.bass as bass
import concourse.tile as tile
from concourse import bacc, mybir
from concourse.bass_utils import run_bass_kernel_spmd
from concourse.masks import make_identity

F32 = mybir.dt.float32
F32R = mybir.dt.float32r
AF = mybir.ActivationFunctionType
ALU = mybir.AluOpType

B, D, H, W, K = 8, 256, 64, 64, 32
HW = H * W          # 4096 spatial positions
NB = D // 128       # 2 channel blocks of 128
NS = HW // 512      # 8 n-slices of 512
NCH = HW // 128     # 32 n-chunks of 128
CW = D + 2          # y_nd chunk width: 256 y + ones + pad (fp32r needs even N)
EPS = 1e-5
N_CORES = 8


def _strided_cols(t, start, step, count, width):
    """AP over columns [start + i*step : start + i*step + width) of a 2D tile."""
    a = t[:, start : start + 1]
    return bass.AP(tensor=a.tensor, offset=a.offset, ap=[a.ap[0], [step, count], [1, width]])


def _build_program(has_bias2):
    nc = bacc.Bacc("TRN2", target_bir_lowering=False, debug=False, num_devices=N_CORES)

    x_d = nc.dram_tensor("x", [D, HW], F32R, kind="ExternalInput").ap()
    wT_d = nc.dram_tensor("wT", [D, D], F32R, kind="ExternalInput").ap()
    ct2_d = nc.dram_tensor("ct2", [D, K], F32, kind="ExternalInput").ap()
    ssc_d = nc.dram_tensor("ssc", [2, K], F32, kind="ExternalInput").ap()
    chv_d = nc.dram_tensor("chv", [D, 4], F32, kind="ExternalInput").ap()
    hwT_d = nc.dram_tensor("hwT", [D, D], F32, kind="ExternalInput").ap()
    ckd_d = nc.dram_tensor("ckd", [K, D], F32, kind="ExternalInput").ap()
    one2_d = nc.dram_tensor("one2", [1, 2], F32R, kind="ExternalInput").ap()
    b2r_d = nc.dram_tensor("b2r", [2, D], F32R, kind="ExternalInput").ap()  # [ones, bias2]
    out_d = nc.dram_tensor("out", [D, HW], F32, kind="ExternalOutput").ap()

    with tile.TileContext(nc) as tc, ExitStack() as ctx:
        sb = ctx.enter_context(tc.tile_pool(name="sb", bufs=1))

        # ---- x load (sync HWDGE queue, 512KB pieces) -------------------
        x_sb = sb.tile([128, NB, HW], F32R)
        for q in range(8):
            cs = slice(q * 512, (q + 1) * 512)
            for c in range(NB):
                nc.sync.dma_start(x_sb[:, c, cs], x_d[c * 128 : (c + 1) * 128, cs])

        # ---- constants (scalar-engine HWDGE queue) ---------------------
        wT = sb.tile([128, NB, D], F32R)         # wT[c,:,o] per c-block
        ct2 = sb.tile([128, NB, K], F32)         # -2*scales[k]*centers[k,d]
        srep = sb.tile([128, K], F32)            # scales replicated over partitions
        sc2row = sb.tile([1, K], F32)            # scales[k]*|c_k|^2
        chv = sb.tile([128, NB, 4], F32)         # [bias2, s1, bb1, -head_b]
        hwT = sb.tile([128, NB, D], F32)         # head_w.T / K
        ckd = sb.tile([32, D], F32)              # centers (k,d)
        one2 = sb.tile([128, 2], F32R)           # fp32r ones (memset can't do f32r)
        b2row = sb.tile([2, D], F32R)            # [ones row, bias2 row] (stem A bias)
        for b in range(NB):
            r = slice(b * 128, (b + 1) * 128)
            nc.scalar.dma_start(wT[:, b, :], wT_d[r, :])
            nc.scalar.dma_start(ct2[:, b, :], ct2_d[r, :])
            nc.scalar.dma_start(chv[:, b, :], chv_d[r, :])
            nc.scalar.dma_start(hwT[:, b, :], hwT_d[r, :])
        nc.scalar.dma_start(srep[:], ssc_d[0:1, :].partition_broadcast(128))
        nc.scalar.dma_start(sc2row[:], ssc_d[1:2, :])
        nc.scalar.dma_start(ckd[:], ckd_d)
        nc.scalar.dma_start(one2[:], one2_d.partition_broadcast(128))
        if has_bias2:
            nc.scalar.dma_start(b2row[:], b2r_d)

        ones_row = sb.tile([1, 128], F32)
        nc.vector.memset(ones_row[:], 1.0)
        ident = sb.tile([32, 32], F32)
        make_identity(nc, ident[:])

        # warm the exp table on ACT early (hidden under the x DMA)
        warm = sb.tile([128, 1], F32)
        nc.vector.memset(warm[:], 0.0)
        nc.scalar.activation(warm[:], warm[:], AF.Exp)

        # ---- big intermediates ----------------------------------------
        y_dn = sb.tile([128, NB, HW], F32)       # relu(W'x): d on partitions
        y_nd = sb.tile([128, NCH * CW], F32R)    # per chunk: 256 y cols + [1, 1]
        ysq = sb.tile([128, NB, HW], F32)        # y_dn^2
        esub = sb.tile([128, NCH * K], F32)      # logits - max
        e_sb = sb.tile([128, NCH * K], F32)      # exp(...)
        a_sb = sb.tile([128, NCH * K], F32R)     # softmax weights
        out_sb = sb.tile([128, NB, HW], F32)

        # ones + pad columns of y_nd (DVE copy from f32r const)
        nc.vector.tensor_copy(
            _strided_cols(y_nd, D, CW, NCH, 2),
            one2[:].rearrange("p (u k) -> p u k", u=1).broadcast_to((128, NCH, 2)))

        with ExitStack() as stem_ctx:
            psB = stem_ctx.enter_context(tc.tile_pool(name="psB", bufs=3, space="PSUM"))
            psA = stem_ctx.enter_context(tc.tile_pool(name="psA", bufs=3, space="PSUM"))
            psL = stem_ctx.enter_context(tc.tile_pool(name="psL", bufs=1, space="PSUM"))

            logits_ps = [psL.tile([128, 512], F32, name=f"logits{i}") for i in range(2)]

            for s in range(NS):
                ns = slice(s * 512, (s + 1) * 512)
                # --- stem B: y_dn[o, ns] = relu(sum_c wT[c,o]x[c,ns] + bias2[o])
                for o in range(NB):
                    pB = psB.tile([128, 512], F32)
                    for c in range(NB):
                        nc.tensor.matmul(
                            pB[:],
                            wT[:, c, o * 128 : (o + 1) * 128],
                            x_sb[:, c, ns],
                            start=(c == 0),
                            stop=(c == NB - 1),
                        )
                    dst = y_dn[:, o, ns]
                    if s % 2 == 0:
                        nc.scalar.activation(dst, pB[:], AF.Relu, bias=chv[:, o, 0:1])
                    else:
                        if has_bias2:
                            nc.vector.tensor_scalar(
                                out=dst, in0=pB[:], scalar1=chv[:, o, 0:1],
                                scalar2=0.0, op0=ALU.add, op1=ALU.max)
                        else:
                            nc.vector.tensor_scalar_max(out=dst, in0=pB[:], scalar1=0.0)

                # --- stem A: y_nd chunk j = relu(x[:,j]^T W' + bias2)
                for half in range(2):
                    pA = psA.tile([128, 512], F32)
                    j0 = 4 * s + 2 * half
                    for ci in range(2):
                        j = j0 + ci
                        jc = slice(j * 128, (j + 1) * 128)
                        po = pA[:, ci * 256 : (ci + 1) * 256]
                        for c in range(NB):
                            nc.tensor.matmul(
                                po,
                                x_sb[:, c, jc],
                                wT[:, c, :],
                                start=(c == 0),
                                stop=(c == NB - 1) and not has_bias2,
                            )
                        if has_bias2:
                            # += 1 * bias2[o]: K=1 matmul, lhsT = ones row
                            nc.tensor.matmul(
                                po, b2row[0:1, 0:128], b2row[1:2, :],
                                start=False, stop=True)
                    dst = _strided_cols(y_nd, j0 * CW, CW, 2, D)
                    nc.scalar.activation(dst, pA[:], AF.Relu)

                # --- squares + logits emitted once a 1024-col quarter of
                #     y_dn is complete (after each odd slice) ------------
                if s % 2 == 1:
                    q = s // 2
                    qs = slice(q * 1024, (q + 1) * 1024)
                    for c in range(NB):
                        eng = nc.gpsimd if q < 2 else nc.vector
                        eng.tensor_mul(ysq[:, c, qs], y_dn[:, c, qs], y_dn[:, c, qs])

                    for sl in (s - 1, s):
                        for j in range(4 * sl, 4 * sl + 4):
                            jc = slice(j * 128, (j + 1) * 128)
                            lp = logits_ps[j // 16]
                            lo = lp[:, (j % 16) * 32 : (j % 16) * 32 + 32]
                            nc.tensor.matmul(lo, ones_row[:], sc2row[:],
                                             start=True, stop=False)
                            for c in range(NB):
                                nc.tensor.matmul(lo, y_dn[:, c, jc], ct2[:, c, :],
                                                 start=False, stop=False)
                            for c in range(NB):
                                nc.tensor.matmul(lo, ysq[:, c, jc], srep[:],
                                                 start=False, stop=(c == NB - 1))

            # ---- softmax over k (32 groups of 32 per partition) --------
            maxt = sb.tile([128, NCH], F32)
            sumt = sb.tile([128, NCH], F32)
            rcp = sb.tile([128, NCH], F32)
            for t in range(2):
                g16 = slice(t * 16, (t + 1) * 16)
                lp3 = logits_ps[t][:].rearrange("p (g k) -> p g k", g=16)
                nc.vector.tensor_reduce(out=maxt[:, g16], in_=lp3,
                                        axis=mybir.AxisListType.X, op=ALU.max)
                mb = maxt[:, g16].rearrange("p (g u) -> p g u", u=1).broadcast_to((128, 16, 32))
                nc.vector.tensor_tensor(
                    out=esub[:, t * 512 : (t + 1) * 512].rearrange("p (g k) -> p g k", g=16),
                    in0=lp3, in1=mb, op=ALU.subtract)
            nc.scalar.activation(e_sb[:], esub[:], AF.Exp)
            nc.vector.tensor_reduce(out=sumt[:], in_=e_sb[:].rearrange("p (g k) -> p g k", g=NCH),
                                    axis=mybir.AxisListType.X, op=ALU.add)
            nc.vector.reciprocal(rcp[:], sumt[:])
            rb = rcp[:].rearrange("p (g u) -> p g u", u=1).broadcast_to((128, NCH, 32))
            nc.vector.tensor_tensor(out=a_sb[:].rearrange("p (g k) -> p g k", g=NCH),
                                    in0=e_sb[:].rearrange("p (g k) -> p g k", g=NCH),
                                    in1=rb, op=ALU.mult)

        # ---- aggregation: psum (32, 258) = a^T [y | 1 | 1] -------------
        with ExitStack() as tail_ctx:
            psG = tail_ctx.enter_context(tc.tile_pool(name="psG", bufs=1, space="PSUM"))
            psT = tail_ctx.enter_context(tc.tile_pool(name="psT", bufs=2, space="PSUM"))
            psH = tail_ctx.enter_context(tc.tile_pool(name="psH", bufs=2, space="PSUM"))

            pagg = psG.tile([32, CW], F32)
            for g in range(NCH):
                nc.tensor.matmul(
                    pagg[:],
                    a_sb[:, g * K : (g + 1) * K],
                    y_nd[:, g * CW : (g + 1) * CW],
                    start=(g == 0), stop=(g == NCH - 1))

            # agg[k,d] = pagg[k,d] - rowsum_a[k] * centers[k,d]
            rsc = sb.tile([32, D], F32)
            nc.vector.tensor_scalar_mul(out=rsc[:], in0=ckd[:], scalar1=pagg[:, D : D + 1])
            agg_sb = sb.tile([32, D], F32)
            nc.vector.tensor_tensor(out=agg_sb[:], in0=pagg[:, 0:D], in1=rsc[:], op=ALU.subtract)

            # BN1 + relu + mean over k  ->  z per d-block
            z_t = sb.tile([128, NB], F32)
            t_sb = sb.tile([128, NB, K], F32)
            for b in range(NB):
                pT = psT.tile([128, 32], F32)
                nc.tensor.transpose(pT[:], agg_sb[:, b * 128 : (b + 1) * 128], ident[:])
                nc.scalar.activation(t_sb[:, b, :], pT[:], AF.Relu,
                                     bias=chv[:, b, 2:3], scale=chv[:, b, 1:2])
                nc.vector.tensor_reduce(out=z_t[:, b : b + 1],
                                        in_=t_sb[:, b, :],
                                        axis=mybir.AxisListType.X, op=ALU.add)

            # head: gate = 1 + sigmoid(head_w @ z + head_b)
            gate = sb.tile([128, NB], F32)
            eg = sb.tile([128, NB], F32)
            for o in range(NB):
                pH = psH.tile([128, 1], F32)
                for c in range(NB):
                    nc.tensor.matmul(pH[:], hwT[:, c, o * 128 : (o + 1) * 128],
                                     z_t[:, c : c + 1],
                                     start=(c == 0), stop=(c == NB - 1))
                # exp(-(v + head_b)) ; gate = 1 + 1/(1+e)
                nc.scalar.activation(eg[:, o : o + 1], pH[:], AF.Exp,
                                     bias=chv[:, o, 3:4], scale=-1.0)
                nc.vector.tensor_scalar_add(out=eg[:, o : o + 1], in0=eg[:, o : o + 1],
                                            scalar1=1.0)
                nc.vector.reciprocal(gate[:, o : o + 1], eg[:, o : o + 1])
                nc.vector.tensor_scalar_add(out=gate[:, o : o + 1],
                                            in0=gate[:, o : o + 1], scalar1=1.0)

            # gating: out = relu(x * gate[d]) ; stream out per 1024-col block
            for o in range(NB):
                for hh in range(2):
                    cs = slice(hh * 2048, (hh + 1) * 2048)
                    if o == 0:
                        nc.scalar.activation(out_sb[:, o, cs], x_sb[:, o, cs],
                                             AF.Relu, scale=gate[:, o : o + 1])
                    else:
                        nc.vector.tensor_scalar(out=out_sb[:, o, cs], in0=x_sb[:, o, cs],
                                                scalar1=gate[:, o : o + 1], scalar2=0.0,
                                                op0=ALU.mult, op1=ALU.max)
                    for q in range(2):
                        qs = slice(hh * 2048 + q * 1024, hh * 2048 + (q + 1) * 1024)
                        nc.sync.dma_start(out_d[o * 128 : (o + 1) * 128, qs], out_sb[:, o, qs])

    nc.compile()
    return nc


_PROGRAM_CACHE = {}


def _get_program(has_bias2):
    key = bool(has_bias2)
    if key not in _PROGRAM_CACHE:
        _PROGRAM_CACHE[key] = _build_program(key)
    return _PROGRAM_CACHE[key]


def _host_params(conv_w, bn2_g, bn2_b, bn2_m, bn2_v, centers, scales,
                 bn1_g, bn1_b, bn1_m, bn1_v, head_w, head_b):
    scale2 = bn2_g / np.sqrt(bn2_v + EPS)
    wT = (conv_w * scale2[:, None]).T.astype(np.float32).copy()      # (c, o)
    bias2 = (bn2_b - bn2_m * scale2).astype(np.float32)
    ct2 = (-2.0 * scales[None, :] * centers.T).astype(np.float32).copy()   # (d, k)
    c2 = (centers * centers).sum(axis=1)
    ssc = np.stack([scales, scales * c2]).astype(np.float32)         # (2, k)
    s1 = bn1_g / np.sqrt(bn1_v + EPS)
    bb1 = bn1_b - bn1_m * s1
    chv = np.stack([bias2, s1.astype(np.float32), bb1.astype(np.float32),
                    (-head_b).astype(np.float32)], axis=1).astype(np.float32)  # (d, 4)
    hwT = (head_w.T / np.float32(K)).astype(np.float32).copy()       # (d, o)
    return wT, bias2, ct2, ssc, chv, hwT


def _ensure_profile_hook():
    """Register the axon NTFF profile hook if the image lacks antenv.axon_hooks."""
    import types

    if "antenv.axon_hooks" in sys.modules:
        return
    try:
        import antenv

        mod = types.ModuleType("antenv.axon_hooks")
        _hook = [None]
        mod.set_axon_ntff_profile_hook = lambda h: _hook.__setitem__(0, h)
        mod.get_axon_ntff_profile_hook = lambda: _hook[0]
        sys.modules["antenv.axon_hooks"] = mod
        antenv.axon_hooks = mod
        from trn_agent_boot.trn_boot import _ntff_profile_via_ctypes

        mod.set_axon_ntff_profile_hook(
            _ntff_profile_via_ctypes("/opt/axon/libaxon_pjrt.so"))
        import concourse.bass_utils as _bu

        _bu.upload_artifacts = lambda d: d  # no artifact store in this container
    except Exception as e:  # profiling is best-effort
        print(f"profile hook setup failed: {e}", file=sys.stderr)


def kernel(x, conv_w, bn2_g, bn2_b, bn2_m, bn2_v, centers, scales,
           bn1_g, bn1_b, bn1_m, bn1_v, head_w, head_b):
    x = np.ascontiguousarray(np.asarray(x, dtype=np.float32))
    wT, bias2, ct2, ssc, chv, hwT = _host_params(
        np.asarray(conv_w, np.float32), np.asarray(bn2_g, np.float32),
        np.asarray(bn2_b, np.float32), np.asarray(bn2_m, np.float32),
        np.asarray(bn2_v, np.float32), np.asarray(centers, np.float32),
        np.asarray(scales, np.float32), np.asarray(bn1_g, np.float32),
        np.asarray(bn1_b, np.float32), np.asarray(bn1_m, np.float32),
        np.asarray(bn1_v, np.float32), np.asarray(head_w, np.float32),
        np.asarray(head_b, np.float32))
    has_bias2 = bool(np.abs(bias2).max() > 0)
    nc = _get_program(has_bias2)

    shared = {
        "wT": wT, "ct2": ct2, "ssc": ssc, "chv": chv, "hwT": hwT,
        "ckd": np.ascontiguousarray(np.asarray(centers, np.float32)),
        "one2": np.ones((1, 2), np.float32),
        "b2r": np.stack([np.ones(D, np.float32), bias2]),
    }
    in_maps = [dict(shared, x=x[b].reshape(D, HW)) for b in range(N_CORES)]

    trace = bool(int(os.environ.get("KERNEL_TRACE", "0")))
    kwargs = {}
    if trace:
        _ensure_profile_hook()
        tdir = os.environ.get("KERNEL_TRACE_DIR")
        if tdir:
            os.makedirs(tdir, exist_ok=True)
            kwargs["tmpdir"] = tdir
    res = run_bass_kernel_spmd(nc, in_maps, list(range(N_CORES)), trace=trace, **kwargs)
    if trace:
        kernel.last_exec_time_ns = res.exec_time_ns
        kernel.last_results = res
    out = np.stack([res.results[b]["out"].reshape(D, H, W) for b in range(N_CORES)])
    return out.astype(np.float32)


# revision 19
# speedup vs baseline: 1.6047x; 1.6047x over previous
"""Trainium2 Bass kernel for nn_EncodingModule2d (vq_codebook).

Pipeline per batch item (pure data parallel, 1 item per NeuronCore, 8 cores):
  stem:   s = conv_w @ x  (1x1 conv as 256x256 matmul over 4096 positions)
          y = relu(BN2(s))                          -- BN folded into weights on host
  vq:     dist2[n,k] = |y_n|^2 - 2<y_n, c_k> + |c_k|^2
          a = softmax_k(scales_k * dist2)
          agg[k,:] = sum_n a[n,k] (y_n - c_k)
  post:   z = mean_k relu(BN1(agg))                 -- BN folded on host
          g = sigmoid(head_w @ z + head_b)
  out:    relu(x + x * g) = relu(x * (1 + g))

The kernel computes the stem in BOTH (d,n) and (n,d) layouts directly from x
(two matmul orientations) because the distance matmul contracts over d while
the aggregation matmul contracts over n; this costs the same PE time as one
stem plus PE transposes but avoids ~20us of PSUM->SBUF copy traffic.

dtype strategy: float32r (1 cyc/row on the PE when N>=256, vs 4 for float32)
for the stem and aggregation matmuls; plain float32 for the N=32 distance
matmuls where fp32r has no speed advantage anyway (and would force rounding
of y). fp32r matmuls require even N, hence the 258-wide aggregation rhs
(256 y columns + ones column + dummy pad column).
"""

import os
import sys

for _p in ("/opt/trn_rl_repo",):
    if _p not in sys.path and os.path.isdir(_p):
        sys.path.insert(0, _p)

from contextlib import ExitStack

import numpy as np

import concourse.bass as bass
import concourse.tile as tile
from concourse import bacc, mybir
from concourse.bass_utils import run_bass_kernel_spmd
from concourse.masks import make_identity

F32 = mybir.dt.float32
F32R = mybir.dt.float32r
AF = mybir.ActivationFunctionType
ALU = mybir.AluOpType

B, D, H, W, K = 8, 256, 64, 64, 32
HW = H * W          # 4096 spatial positions
NB = D // 128       # 2 channel blocks of 128
NS = HW // 512      # 8 n-slices of 512
NCH = HW // 128     # 32 n-chunks of 128
CW = D + 2          # y_nd chunk width: 256 y + ones + pad (fp32r needs even N)
EPS = 1e-5
N_CORES = 8


def _strided_cols(t, start, step, count, width):
    """AP over columns [start + i*step : start + i*step + width) of a 2D tile."""
    a = t[:, start : start + 1]
    return bass.AP(tensor=a.tensor, offset=a.offset, ap=[a.ap[0], [step, count], [1, width]])


def _build_program(has_bias2):
    nc = bacc.Bacc("TRN2", target_bir_lowering=False, debug=False, num_devices=N_CORES)

    x_d = nc.dram_tensor("x", [D, HW], F32R, kind="ExternalInput").ap()
    wT_d = nc.dram_tensor("wT", [D, D], F32R, kind="ExternalInput").ap()
    ct2_d = nc.dram_tensor("ct2", [D, K], F32R, kind="ExternalInput").ap()
    ssc_d = nc.dram_tensor("ssc", [1, K], F32R, kind="ExternalInput").ap()
    scc_d = nc.dram_tensor("scc", [K, 1], F32, kind="ExternalInput").ap()
    chv_d = nc.dram_tensor("chv", [D, 4], F32, kind="ExternalInput").ap()
    hwT_d = nc.dram_tensor("hwT", [D, D], F32, kind="ExternalInput").ap()
    ckd_d = nc.dram_tensor("ckd", [K, D], F32, kind="ExternalInput").ap()
    one2_d = nc.dram_tensor("one2", [1, 2], F32R, kind="ExternalInput").ap()
    b2r_d = nc.dram_tensor("b2r", [2, D], F32R, kind="ExternalInput").ap()  # [ones, bias2]
    out_d = nc.dram_tensor("out", [D, HW], F32, kind="ExternalOutput").ap()

    with tile.TileContext(nc) as tc, ExitStack() as ctx:
        sb = ctx.enter_context(tc.tile_pool(name="sb", bufs=1))

        # ---- x load (sync HWDGE queue, 512KB pieces) -------------------
        x_sb = sb.tile([128, NB, HW], F32R)
        for q in range(8):
            cs = slice(q * 512, (q + 1) * 512)
            for c in range(NB):
                nc.sync.dma_start(x_sb[:, c, cs], x_d[c * 128 : (c + 1) * 128, cs])

        # ---- constants (scalar-engine HWDGE queue) ---------------------
        wT = sb.tile([128, NB, D], F32R)         # wT[c,:,o] per c-block
        ct2 = sb.tile([128, NB, K], F32R)        # -2*scales[k]*centers[k,d]
        srep = sb.tile([128, K], F32R)           # scales replicated over partitions
        sc2col = sb.tile([32, 1], F32)           # scales[k]*|c_k|^2 (bias column)
        chv = sb.tile([128, NB, 4], F32)         # [bias2, s1, bb1, -head_b]
        hwT = sb.tile([128, NB, D], F32)         # head_w.T / K
        ckd = sb.tile([32, D], F32)              # centers (k,d)
        one2 = sb.tile([128, 2], F32R)           # fp32r ones (memset can't do f32r)
        b2row = sb.tile([2, D], F32R)            # [ones row, bias2 row] (stem A bias)
        for b in range(NB):
            r = slice(b * 128, (b + 1) * 128)
            nc.scalar.dma_start(wT[:, b, :], wT_d[r, :])
            nc.scalar.dma_start(ct2[:, b, :], ct2_d[r, :])
            nc.scalar.dma_start(chv[:, b, :], chv_d[r, :])
            nc.scalar.dma_start(hwT[:, b, :], hwT_d[r, :])
        nc.scalar.dma_start(srep[:], ssc_d.partition_broadcast(128))
        nc.scalar.dma_start(sc2col[:], scc_d)
        nc.scalar.dma_start(ckd[:], ckd_d)
        nc.scalar.dma_start(one2[:], one2_d.partition_broadcast(128))
        if has_bias2:
            nc.scalar.dma_start(b2row[:], b2r_d)

        ident = sb.tile([32, 32], F32)
        make_identity(nc, ident[:])

        # warm the exp table on ACT early (hidden under the x DMA)
        warm = sb.tile([128, 1], F32)
        nc.vector.memset(warm[:], 0.0)
        nc.scalar.activation(warm[:], warm[:], AF.Exp)

        # ---- big intermediates ----------------------------------------
        y_dn = sb.tile([128, NB, HW], F32R)      # relu(W'x): d on partitions
        y_nd = sb.tile([128, NCH * CW], F32R)    # per chunk: 256 y cols + [1, 1]
        ysq = sb.tile([128, NB, HW], F32R)       # y_dn^2
        lkn = sb.tile([32, HW], F32)             # logits in (k, n) layout
        esub = sb.tile([128, NCH * K], F32)      # logits - max
        e_sb = sb.tile([128, NCH * K], F32)      # exp(...)
        a_sb = sb.tile([128, NCH * K], F32R)     # softmax weights
        out_sb = sb.tile([128, NB, HW], F32)

        # ones + pad columns of y_nd (DVE copy from f32r const)
        nc.vector.tensor_copy(
            _strided_cols(y_nd, D, CW, NCH, 2),
            one2[:].rearrange("p (u k) -> p u k", u=1).broadcast_to((128, NCH, 2)))

        with ExitStack() as stem_ctx:
            psB = stem_ctx.enter_context(tc.tile_pool(name="psB", bufs=2, space="PSUM"))
            psA = stem_ctx.enter_context(tc.tile_pool(name="psA", bufs=2, space="PSUM"))
            psK = stem_ctx.enter_context(tc.tile_pool(name="psK", bufs=2, space="PSUM"))
            psL = stem_ctx.enter_context(tc.tile_pool(name="psL", bufs=1, space="PSUM"))

            logits_ps = [psL.tile([128, 512], F32, name=f"logits{i}") for i in range(2)]

            for s in range(NS):
                ns = slice(s * 512, (s + 1) * 512)
                # --- stem B: y_dn[o, ns] = relu(sum_c wT[c,o]x[c,ns] + bias2[o])
                for o in range(NB):
                    pB = psB.tile([128, 512], F32)
                    for c in range(NB):
                        nc.tensor.matmul(
                            pB[:],
                            wT[:, c, o * 128 : (o + 1) * 128],
                            x_sb[:, c, ns],
                            start=(c == 0),
                            stop=(c == NB - 1),
                        )
                    dst = y_dn[:, o, ns]
                    if s % 2 == 0:
                        nc.scalar.activation(dst, pB[:], AF.Relu, bias=chv[:, o, 0:1])
                    else:
                        if has_bias2:
                            nc.vector.tensor_scalar(
                                out=dst, in0=pB[:], scalar1=chv[:, o, 0:1],
                                scalar2=0.0, op0=ALU.add, op1=ALU.max)
                        else:
                            nc.vector.tensor_scalar_max(out=dst, in0=pB[:], scalar1=0.0)

                # --- stem A: y_nd chunk j = relu(x[:,j]^T W' + bias2)
                for half in range(2):
                    pA = psA.tile([128, 512], F32)
                    j0 = 4 * s + 2 * half
                    for ci in range(2):
                        j = j0 + ci
                        jc = slice(j * 128, (j + 1) * 128)
                        po = pA[:, ci * 256 : (ci + 1) * 256]
                        for c in range(NB):
                            nc.tensor.matmul(
                                po,
                                x_sb[:, c, jc],
                                wT[:, c, :],
                                start=(c == 0),
                                stop=(c == NB - 1) and not has_bias2,
                            )
                        if has_bias2:
                            # += 1 * bias2[o]: K=1 matmul, lhsT = ones row
                            nc.tensor.matmul(
                                po, b2row[0:1, 0:128], b2row[1:2, :],
                                start=False, stop=True)
                    dst = _strided_cols(y_nd, j0 * CW, CW, 2, D)
                    nc.scalar.activation(dst, pA[:], AF.Relu)

                # --- squares + logits emitted once a 1024-col quarter of
                #     y_dn is complete (after each odd slice) ------------
                # logits in (k, n) orientation so the 128x32 constants stay
                # (cheaply re-)loaded as stationary weights:
                #   lkn[k, n] = sum_d ct2[d,k] y[d,n] + sum_d srep[d,k] ysq[d,n]
                #             = -2 s_k <y_n, c_k> + s_k |y_n|^2
                # + sc2[k] added as a per-partition bias in the PSUM->SBUF copy,
                # then 32x128 blocks are PE-transposed into (n, k) psum banks.
                if s % 2 == 1:
                    q = s // 2
                    qs = slice(q * 1024, (q + 1) * 1024)
                    for c in range(NB):
                        eng = nc.gpsimd if q < 2 else nc.vector
                        eng.tensor_mul(ysq[:, c, qs], y_dn[:, c, qs], y_dn[:, c, qs])

                    for si, sl in enumerate((s - 1, s)):
                        pK = psK.tile([32, 512], F32)
                        nsl = slice(sl * 512, (sl + 1) * 512)
                        nc.tensor.matmul(pK[:], ct2[:, 0, :], y_dn[:, 0, nsl],
                                         start=True, stop=False)
                        nc.tensor.matmul(pK[:], ct2[:, 1, :], y_dn[:, 1, nsl],
                                         start=False, stop=False)
                        nc.tensor.matmul(pK[:], srep[:], ysq[:, 0, nsl],
                                         start=False, stop=False)
                        nc.tensor.matmul(pK[:], srep[:], ysq[:, 1, nsl],
                                         start=False, stop=True)
                        dst = lkn[:, nsl]
                        if si == 0:
                            nc.scalar.activation(dst, pK[:], AF.Identity, bias=sc2col[:])
                        else:
                            nc.vector.tensor_scalar_add(out=dst, in0=pK[:],
                                                        scalar1=sc2col[:])
                    for j in range(4 * (s - 1), 4 * s + 4):
                        lp = logits_ps[j // 16]
                        nc.tensor.transpose(lp[:, (j % 16) * 32 : (j % 16) * 32 + 32],
                                            lkn[:, j * 128 : (j + 1) * 128], ident[:])

            # ---- softmax over k (32 groups of 32 per partition) --------
            maxt = sb.tile([128, NCH], F32)
            sumt = sb.tile([128, NCH], F32)
            rcp = sb.tile([128, NCH], F32)
            for t in range(2):
                g16 = slice(t * 16, (t + 1) * 16)
                lp3 = logits_ps[t][:].rearrange("p (g k) -> p g k", g=16)
                nc.vector.tensor_reduce(out=maxt[:, g16], in_=lp3,
                                        axis=mybir.AxisListType.X, op=ALU.max)
                mb = maxt[:, g16].rearrange("p (g u) -> p g u", u=1).broadcast_to((128, 16, 32))
                nc.vector.tensor_tensor(
                    out=esub[:, t * 512 : (t + 1) * 512].rearrange("p (g k) -> p g k", g=16),
                    in0=lp3, in1=mb, op=ALU.subtract)
            nc.scalar.activation(e_sb[:], esub[:], AF.Exp)
            nc.vector.tensor_reduce(out=sumt[:], in_=e_sb[:].rearrange("p (g k) -> p g k", g=NCH),
                                    axis=mybir.AxisListType.X, op=ALU.add)
            nc.vector.reciprocal(rcp[:], sumt[:])
            rb = rcp[:].rearrange("p (g u) -> p g u", u=1).broadcast_to((128, NCH, 32))
            nc.vector.tensor_tensor(out=a_sb[:].rearrange("p (g k) -> p g k", g=NCH),
                                    in0=e_sb[:].rearrange("p (g k) -> p g k", g=NCH),
                                    in1=rb, op=ALU.mult)

        # ---- aggregation: psum (32, 258) = a^T [y | 1 | 1] -------------
        with ExitStack() as tail_ctx:
            psG = tail_ctx.enter_context(tc.tile_pool(name="psG", bufs=1, space="PSUM"))
            psT = tail_ctx.enter_context(tc.tile_pool(name="psT", bufs=2, space="PSUM"))
            psH = tail_ctx.enter_context(tc.tile_pool(name="psH", bufs=2, space="PSUM"))

            pagg = psG.tile([32, CW], F32)
            for g in range(NCH):
                nc.tensor.matmul(
                    pagg[:],
                    a_sb[:, g * K : (g + 1) * K],
                    y_nd[:, g * CW : (g + 1) * CW],
                    start=(g == 0), stop=(g == NCH - 1))

            # agg[k,d] = pagg[k,d] - rowsum_a[k] * centers[k,d]
            rsc = sb.tile([32, D], F32)
            nc.vector.tensor_scalar_mul(out=rsc[:], in0=ckd[:], scalar1=pagg[:, D : D + 1])
            agg_sb = sb.tile([32, D], F32)
            nc.vector.tensor_tensor(out=agg_sb[:], in0=pagg[:, 0:D], in1=rsc[:], op=ALU.subtract)

            # BN1 + relu + mean over k  ->  z per d-block
            z_t = sb.tile([128, NB], F32)
            t_sb = sb.tile([128, NB, K], F32)
            for b in range(NB):
                pT = psT.tile([128, 32], F32)
                nc.tensor.transpose(pT[:], agg_sb[:, b * 128 : (b + 1) * 128], ident[:])
                nc.scalar.activation(t_sb[:, b, :], pT[:], AF.Relu,
                                     bias=chv[:, b, 2:3], scale=chv[:, b, 1:2])
                nc.vector.tensor_reduce(out=z_t[:, b : b + 1],
                                        in_=t_sb[:, b, :],
                                        axis=mybir.AxisListType.X, op=ALU.add)

            # head: gate = 1 + sigmoid(head_w @ z + head_b)
            gate = sb.tile([128, NB], F32)
            eg = sb.tile([128, NB], F32)
            for o in range(NB):
                pH = psH.tile([128, 1], F32)
                for c in range(NB):
                    nc.tensor.matmul(pH[:], hwT[:, c, o * 128 : (o + 1) * 128],
                                     z_t[:, c : c + 1],
                                     start=(c == 0), stop=(c == NB - 1))
                # exp(-(v + head_b)) ; gate = 1 + 1/(1+e)
                nc.scalar.activation(eg[:, o : o + 1], pH[:], AF.Exp,
                                     bias=chv[:, o, 3:4], scale=-1.0)
                nc.vector.tensor_scalar_add(out=eg[:, o : o + 1], in0=eg[:, o : o + 1],
                                            scalar1=1.0)
                nc.vector.reciprocal(gate[:, o : o + 1], eg[:, o : o + 1])
                nc.vector.tensor_scalar_add(out=gate[:, o : o + 1],
                                            in0=gate[:, o : o + 1], scalar1=1.0)

            # gating: out = relu(x * gate[d]) ; stream out per 1024-col block
            for o in range(NB):
                for hh in range(2):
                    cs = slice(hh * 2048, (hh + 1) * 2048)
                    if o == 0:
                        nc.scalar.activation(out_sb[:, o, cs], x_sb[:, o, cs],
                                             AF.Relu, scale=gate[:, o : o + 1])
                    else:
                        nc.vector.tensor_scalar(out=out_sb[:, o, cs], in0=x_sb[:, o, cs],
                                                scalar1=gate[:, o : o + 1], scalar2=0.0,
                                                op0=ALU.mult, op1=ALU.max)
                    for q in range(2):
                        qs = slice(hh * 2048 + q * 1024, hh * 2048 + (q + 1) * 1024)
                        nc.sync.dma_start(out_d[o * 128 : (o + 1) * 128, qs], out_sb[:, o, qs])

    nc.compile()
    return nc


_PROGRAM_CACHE = {}


def _get_program(has_bias2):
    key = bool(has_bias2)
    if key not in _PROGRAM_CACHE:
        _PROGRAM_CACHE[key] = _build_program(key)
    return _PROGRAM_CACHE[key]


def _host_params(conv_w, bn2_g, bn2_b, bn2_m, bn2_v, centers, scales,
                 bn1_g, bn1_b, bn1_m, bn1_v, head_w, head_b):
    scale2 = bn2_g / np.sqrt(bn2_v + EPS)
    wT = (conv_w * scale2[:, None]).T.astype(np.float32).copy()      # (c, o)
    bias2 = (bn2_b - bn2_m * scale2).astype(np.float32)
    ct2 = (-2.0 * scales[None, :] * centers.T).astype(np.float32).copy()   # (d, k)
    c2 = (centers * centers).sum(axis=1)
    ssc = scales.reshape(1, K).astype(np.float32)                    # (1, k)
    scc = (scales * c2).reshape(K, 1).astype(np.float32)             # (k, 1)
    s1 = bn1_g / np.sqrt(bn1_v + EPS)
    bb1 = bn1_b - bn1_m * s1
    chv = np.stack([bias2, s1.astype(np.float32), bb1.astype(np.float32),
                    (-head_b).astype(np.float32)], axis=1).astype(np.float32)  # (d, 4)
    hwT = (head_w.T / np.float32(K)).astype(np.float32).copy()       # (d, o)
    return wT, bias2, ct2, ssc, scc, chv, hwT


def _ensure_profile_hook():
    """Register the axon NTFF profile hook if the image lacks antenv.axon_hooks."""
    import types

    if "antenv.axon_hooks" in sys.modules:
        return
    try:
        import antenv

        mod = types.ModuleType("antenv.axon_hooks")
        _hook = [None]
        mod.set_axon_ntff_profile_hook = lambda h: _hook.__setitem__(0, h)
        mod.get_axon_ntff_profile_hook = lambda: _hook[0]
        sys.modules["antenv.axon_hooks"] = mod
        antenv.axon_hooks = mod
        from trn_agent_boot.trn_boot import _ntff_profile_via_ctypes

        mod.set_axon_ntff_profile_hook(
            _ntff_profile_via_ctypes("/opt/axon/libaxon_pjrt.so"))
        import concourse.bass_utils as _bu

        _bu.upload_artifacts = lambda d: d  # no artifact store in this container
    except Exception as e:  # profiling is best-effort
        print(f"profile hook setup failed: {e}", file=sys.stderr)


def kernel(x, conv_w, bn2_g, bn2_b, bn2_m, bn2_v, centers, scales,
           bn1_g, bn1_b, bn1_m, bn1_v, head_w, head_b):
    x = np.ascontiguousarray(np.asarray(x, dtype=np.float32))
    wT, bias2, ct2, ssc, scc, chv, hwT = _host_params(
        np.asarray(conv_w, np.float32), np.asarray(bn2_g, np.float32),
        np.asarray(bn2_b, np.float32), np.asarray(bn2_m, np.float32),
        np.asarray(bn2_v, np.float32), np.asarray(centers, np.float32),
        np.asarray(scales, np.float32), np.asarray(bn1_g, np.float32),
        np.asarray(bn1_b, np.float32), np.asarray(bn1_m, np.float32),
        np.asarray(bn1_v, np.float32), np.asarray(head_w, np.float32),
        np.asarray(head_b, np.float32))
    has_bias2 = bool(np.abs(bias2).max() > 0)
    nc = _get_program(has_bias2)

    shared = {
        "wT": wT, "ct2": ct2, "ssc": ssc, "scc": scc, "chv": chv, "hwT": hwT,
        "ckd": np.ascontiguousarray(np.asarray(centers, np.float32)),
        "one2": np.ones((1, 2), np.float32),
        "b2r": np.stack([np.ones(D, np.float32), bias2]),
    }
    in_maps = [dict(shared, x=x[b].reshape(D, HW)) for b in range(N_CORES)]

    trace = bool(int(os.environ.get("KERNEL_TRACE", "0")))
    kwargs = {}
    if trace:
        _ensure_profile_hook()
        tdir = os.environ.get("KERNEL_TRACE_DIR")
        if tdir:
            os.makedirs(tdir, exist_ok=True)
            kwargs["tmpdir"] = tdir
    res = run_bass_kernel_spmd(nc, in_maps, list(range(N_CORES)), trace=trace, **kwargs)
    if trace:
        kernel.last_exec_time_ns = res.exec_time_ns
        kernel.last_results = res
    out = np.stack([res.results[b]["out"].reshape(D, H, W) for b in range(N_CORES)])
    return out.astype(np.float32)
